# revision 50
# baseline (speedup 1.0000x reference)
"""GQA attention prefill (B=4, S=2048, D=4096, 32 q-heads / 8 kv-heads, rotary,
causal) on 8 TRN2 NeuronCores.

Sharding: token-parallel attention + tensor-parallel K/V projection.

Token side: core c handles batch c//2 and two 512-token chunks of its
sequence: chunks {0,3} for even cores, {1,2} for odd cores (zigzag splits the
causal triangle evenly). Each core computes the full Q projection for its
tokens, attention for all 32 heads over its tokens, and the output projection
for its tokens. Outputs are disjoint token slices, gathered on host.

K/V side: the 8 kv heads are tensor-parallel across the core PAIR that shares
a batch — role r = c%2 computes only kv heads {4r..4r+3} over the full
2048-token prefix (halves the K/V projection PE work). Each K/V tile is
written twice into a GLOBAL-layout DRAM buffer — once at the role-0 position
scaled by m0, once at the role-1 position scaled by m1, where (m0,m1) are
per-core {0,1} input scalars — so every core produces the same-layout buffer
with zeros in the partner's slots. The buffer is duplicated into two halves
(one DRAM->DRAM copy) and a pairwise ReduceScatter(add) (replica groups
[[0,1],[2,3],[4,5],[6,7]]) hands BOTH cores the fully merged global K/V at
the same address: zero+x = x, and both scatter halves are identical. The
SPMD program stays identical on all cores — no role-dependent addresses.

The collective fires right after the K/V pass and flies during the Q
projection (~380us of PE work), so its ~225us latency is hidden.

DMA-queue routing (each queue's sequencer serializes its DMAs through the
transfer, so dispatch is a per-queue resource):
  - sync (SP):    strips, odd q/v weights, v4/qt4/mask loads, oT/out writes
  - gpsimd (Pool): even q/v weights, kw, ow streams, the collective
  - scalar (Act):  K/V global-buffer writes, the D2D duplicate, qrot writes
                   (Act is otherwise idle outside attention)
  - vector (DVE):  kT loads for attention

Precision: projections bf16 x bf16 -> f32 PSUM; rotary applied on PSUM f32,
K/Q written back as bf16; softmax denominator via ones-vector matmul in PSUM
f32; AV and the output projection bf16.

Layout conventions:
  - activations for QK^T are kept transposed: [head_dim (partitions), tokens]
  - rotary pairs are de-interleaved (even dims -> partitions 0-63, odd ->
    64-127) via a host-side permutation of the qw/kw rows.
  - attention runs scores-transposed: ST[key, query] = kT.T @ qT, softmax over
    the partition (key) axis, denominator via ones-vector matmul, no
    max-subtraction (scores are O(1)).
"""

import numpy as np
import ml_dtypes

import concourse.bacc as bacc
import concourse.bass as bass
import concourse.tile as tile
from concourse import library_config, mybir
from concourse.bass_utils import run_bass_kernel_spmd

F32 = mybir.dt.float32
F32R = mybir.dt.float32r
BF16 = mybir.dt.bfloat16
EXP = mybir.ActivationFunctionType.Exp
ADD = mybir.AluOpType.add
MULT = mybir.AluOpType.mult

B, S, D = 4, 2048, 4096
QH, KVH, HEAD = 32, 8, 128
P = 128
CH = 512                # token chunk (= query tile)
NCH = S // CH           # 4 chunks per sequence
ND = D // P             # 32 d-tiles
LKV = 4                 # kv heads per core (tensor-parallel within pair)
NCORES = 8
NKB = (8, 16)           # key-blocks per query slot (padded, uniform)
SCALE = 1.0 / np.sqrt(HEAD)
BF = ml_dtypes.bfloat16
PAIRS = [[0, 1], [2, 3], [4, 5], [6, 7]]

_CACHE = {}


def _build():
    nc = bacc.Bacc("TRN2", target_bir_lowering=False, debug=False, num_devices=NCORES)

    # ---- per-core external inputs ----
    own = nc.dram_tensor("own_strips", [2, P, ND, CH], BF16, kind="ExternalInput")
    pref = nc.dram_tensor("in_strips", [NCH, P, ND, CH], BF16, kind="ExternalInput")
    qwT = nc.dram_tensor("qwT", [QH, P, ND, P], BF16, kind="ExternalInput")
    kwT = nc.dram_tensor("kwT_loc", [LKV, P, ND, P], BF16, kind="ExternalInput")
    vwT = nc.dram_tensor("vwT_loc", [ND, P, LKV * HEAD], BF16, kind="ExternalInput")
    owT = nc.dram_tensor("owT", [8, ND, P, 512], BF16, kind="ExternalInput")
    cos_own = nc.dram_tensor("cos_own", [64, 2, CH], F32, kind="ExternalInput")
    sin_own = nc.dram_tensor("sin_own", [64, 2, CH], F32, kind="ExternalInput")
    cos_all = nc.dram_tensor("cos_all", [64, S], F32, kind="ExternalInput")
    sin_all = nc.dram_tensor("sin_all", [64, S], F32, kind="ExternalInput")
    qbT = nc.dram_tensor("qbT", [P, QH], F32, kind="ExternalInput")
    kbT = nc.dram_tensor("kbT_loc", [P, LKV], F32, kind="ExternalInput")
    vbm = nc.dram_tensor("vbm", [2 * LKV * HEAD], F32, kind="ExternalInput")
    ob = nc.dram_tensor("ob", [D], F32, kind="ExternalInput")
    masks = nc.dram_tensor("masks", [2, 8, P, CH], BF16, kind="ExternalInput")
    ones = nc.dram_tensor("ones", [P, P], BF16, kind="ExternalInput")
    mm = nc.dram_tensor("mm", [P, 2], F32, kind="ExternalInput")

    # ---- internal DRAM ----
    # Global K/V layout (kind, row, P, 1024):
    #   kind 0 = kT: kv head j (GLOBAL), chunk tg -> row 2j + tg//2,
    #       cols (tg%2)*512.  (kT_g[j] = [128 hd, 2048 keys] as 2 rows)
    #   kind 1 = v: key-block kb -> row kb, cols j*128+hd (j GLOBAL kv head)
    # cc_in has two identical halves of this layout (masked: partner slots
    # zero); ReduceScatter(add) over the pair yields the merged layout cc_m
    # on both cores.
    cc_in = nc.dram_tensor("cc_in", [2, 2, 2 * KVH, P, 1024], BF16)
    cc_m = nc.dram_tensor("cc_m", [2, 2 * KVH, P, 1024], BF16)
    qT_i = nc.dram_tensor("qT_i", [2, QH, P, CH], BF16)
    oT_i = nc.dram_tensor("oT_i", [2, QH, P, CH], BF16)

    out = nc.dram_tensor("out", [8, P, D], F32, kind="ExternalOutput")

    with tile.TileContext(nc) as tc:
        nc.gpsimd.load_library(library_config.lib)
        with (
            tc.tile_pool(name="const", bufs=1) as const,
            tc.tile_pool(name="ev", bufs=2) as evpool,
            tc.tile_pool(name="rt", bufs=4) as rtpool,
            tc.tile_pool(name="ps", bufs=8, space="PSUM") as pspool,
        ):
            kbT_e = const.tile([64, LKV], F32, tag="kbte")
            kbT_o = const.tile([64, LKV], F32, tag="kbto")
            nc.scalar.dma_start(out=kbT_e[:], in_=kbT[0:64, :])
            nc.scalar.dma_start(out=kbT_o[:], in_=kbT[64:P, :])
            # all-ones stationary matrix: the denominator matmul writes the
            # key-sum replicated across ALL partitions, so the softmax tail
            # needs no partition_broadcast (no Pool-ring round trip).
            ones_mat = const.tile([P, P], BF16, tag="oc")
            nc.scalar.dma_start(out=ones_mat[:], in_=ones[:])
            mm_sb = const.tile([P, 2], F32, tag="mm")
            nc.scalar.dma_start(out=mm_sb[:], in_=mm[:])

            def rotary_evict(ps, dst, cos_ap, sin_ap, be, bo):
                """dst[0:64]=(pe+be)*cos-(po+bo)*sin; dst[64:128]=(pe+be)*sin+(po+bo)*cos"""
                pe, po = ps[0:64, :], ps[64:128, :]
                t1 = rtpool.tile([64, CH], F32, tag="rt", name="t1")
                t2 = rtpool.tile([64, CH], F32, tag="rt", name="t2")
                nc.vector.scalar_tensor_tensor(t1[:], pe, be, cos_ap, ADD, MULT)
                nc.vector.scalar_tensor_tensor(t2[:], po, bo, sin_ap, ADD, MULT)
                nc.vector.tensor_sub(dst[0:64, :], t1[:], t2[:])
                t3 = rtpool.tile([64, CH], F32, tag="rt", name="t3")
                t4 = rtpool.tile([64, CH], F32, tag="rt", name="t4")
                nc.vector.scalar_tensor_tensor(t3[:], pe, be, sin_ap, ADD, MULT)
                nc.vector.scalar_tensor_tensor(t4[:], po, bo, cos_ap, ADD, MULT)
                nc.vector.tensor_add(dst[64:128, :], t3[:], t4[:])

            p01_cm = tc.tile_pool(name="strip", bufs=3)
            strip_pool = p01_cm.__enter__()
            kw_cm = tc.tile_pool(name="kw", bufs=1)
            kwpool = kw_cm.__enter__()
            kw_tiles = {}

            # ============ P1: K/V projection, LOCAL kv heads only ============
            with tc.tile_pool(name="p1c", bufs=1) as p1c:
                cos_all_sb = p1c.tile([64, S], F32, tag="cosa")
                sin_all_sb = p1c.tile([64, S], F32, tag="sina")
                nc.sync.dma_start(out=cos_all_sb[:], in_=cos_all[:])
                nc.sync.dma_start(out=sin_all_sb[:], in_=sin_all[:])
                vbm_sb = p1c.tile([P, 2 * LKV * HEAD], F32, tag="vbm")
                nc.sync.dma_start(
                    out=vbm_sb[:], in_=vbm.ap()[None, :].partition_broadcast(P)
                )
                with tc.tile_pool(name="wb", bufs=6) as wbpool, \
                     tc.tile_pool(name="stg", bufs=1) as stgpool:
                    for pr in range(2):
                        strips = []
                        for i in range(2):
                            st = strip_pool.tile(
                                [P, ND, CH], BF16, tag="strip", name=f"strip{pr}_{i}"
                            )
                            # pr0's second strip rides the Act ring's idle
                            # start so both strips are up before the K pass;
                            # pr0's first strip loads in two halves so the
                            # first matmul can start after half the transfer
                            if pr == 0 and i == 0:
                                nc.sync.dma_start(
                                    out=st[:, 0 : ND // 2, :],
                                    in_=pref[0, :, 0 : ND // 2, :],
                                )
                                nc.sync.dma_start(
                                    out=st[:, ND // 2 :, :],
                                    in_=pref[0, :, ND // 2 :, :],
                                )
                            else:
                                eng = nc.scalar if (pr == 0 and i == 1) else nc.sync
                                eng.dma_start(out=st[:], in_=pref[2 * pr + i])
                            strips.append(st)
                        # K-pass (weight-stationary, out = kT [f, t]), 4 local
                        # heads. ts outer / kv inner: the 4 heads of one chunk
                        # land in consecutive global rows tg*4..tg*4+3, staged
                        # in SBUF and written per chunk as ONE 4-row DMA per
                        # scatter half (8x fewer ring-dispatch serializations).
                        for ts in range(2):
                            tg = 2 * pr + ts
                            ksg = stgpool.tile([P, LKV, 2, CH], BF16, tag="ksg",
                                               name="ksg")
                            for kv in range(LKV):
                                if kv in kw_tiles:
                                    w = kw_tiles[kv]
                                else:
                                    w = kwpool.tile([P, ND, P], BF16, tag=f"kw{kv}",
                                                    name=f"kw{kv}")
                                    nc.gpsimd.dma_start(out=w[:], in_=kwT[kv])
                                    kw_tiles[kv] = w
                                ps = pspool.tile([P, CH], F32, tag="ps", name="ps_k")
                                for dt in range(ND):
                                    nc.tensor.matmul(
                                        ps[:], lhsT=w[:, dt, :], rhs=strips[ts][:, dt, :],
                                        start=(dt == 0), stop=(dt == ND - 1),
                                    )
                                krot = evpool.tile([P, CH], F32, tag="ev", name="krot")
                                rotary_evict(
                                    ps, krot,
                                    cos_all_sb[:, tg * CH : (tg + 1) * CH],
                                    sin_all_sb[:, tg * CH : (tg + 1) * CH],
                                    kbT_e[:, kv : kv + 1], kbT_o[:, kv : kv + 1],
                                )
                                for q in range(2):
                                    nc.vector.tensor_scalar_mul(
                                        ksg[:, kv, q, :], krot[:], mm_sb[:, q : q + 1]
                                    )
                            for half, eng in ((0, nc.scalar), (1, nc.gpsimd)):
                                eng.dma_start(
                                    out=cc_in[half, 0, tg * LKV : (tg + 1) * LKV]
                                    .rearrange("r p t -> p r t"),
                                    in_=ksg[:],
                                )
                        # V-pass (input-stationary, out = v [t, hd]), 4 local heads
                        psv = [
                            pspool.tile([P, 512], F32, tag="ps", name=f"psv{i}")
                            for i in range(8)
                        ]
                        for dt in range(ND):
                            vw = wbpool.tile([P, 512], BF16, tag="wb", name="vw")
                            eng = nc.gpsimd if dt % 2 == 0 else nc.sync
                            eng.dma_start(out=vw[:], in_=vwT[dt])
                            for ts in range(2):
                                for tt in range(4):
                                    nc.tensor.matmul(
                                        psv[ts * 4 + tt][:],
                                        lhsT=strips[ts][:, dt, tt * P : (tt + 1) * P],
                                        rhs=vw[:],
                                        start=(dt == 0), stop=(dt == ND - 1),
                                    )
                        for ts in range(2):
                            kb0 = (2 * pr + ts) * 4
                            vsg = stgpool.tile([P, 4, 1024], BF16, tag="vsg", name="vsg")
                            for tt in range(4):
                                # masked double-write into both role column
                                # halves of the staged global v rows
                                for q in range(2):
                                    nc.vector.scalar_tensor_tensor(
                                        vsg[:, tt, q * 512 : (q + 1) * 512],
                                        psv[ts * 4 + tt][:],
                                        mm_sb[:, q : q + 1],
                                        vbm_sb[:, q * 512 : (q + 1) * 512],
                                        MULT, ADD,
                                    )
                            for half, eng in ((0, nc.scalar), (1, nc.gpsimd)):
                                eng.dma_start(
                                    out=cc_in[half, 1, kb0 : kb0 + 4]
                                    .rearrange("r p t -> p r t"),
                                    in_=vsg[:],
                                )

            kw_cm.__exit__(None, None, None)
            p01_cm.__exit__(None, None, None)

            # Pairwise ReduceScatter(add): both scatter halves are written
            # identically by the evictions, so every core receives the merged
            # global K/V at cc_m. Emitted before any Q-projection instruction
            # so the scheduler places it at the head of the Pool ring.
            nc.gpsimd.collective_compute(
                "ReduceScatter",
                mybir.AluOpType.add,
                replica_groups=PAIRS,
                ins=[cc_in.ap()],
                outs=[cc_m.ap()],
            )

            kv_cm = tc.tile_pool(name="kvS", bufs=1)
            kvpool = kv_cm.__enter__()
            qt_cm = tc.tile_pool(name="qtS", bufs=2)
            qtpool = qt_cm.__enter__()
            pt_cm = tc.tile_pool(name="ptS", bufs=3)
            ptpool = pt_cm.__enter__()
            r_cm = tc.tile_pool(name="rS", bufs=2)
            rpool = r_cm.__enter__()
            qw_cm = tc.tile_pool(name="qw", bufs=3)
            wpool = qw_cm.__enter__()

            # ============ P0: Q projection + rotary -> qT_i (bf16) ============
            # Slot 0 + 24 heads of slot 1 emitted eagerly (covers the
            # collective); the last 8 heads are woven into attention slot 0.
            p0s_cm = tc.tile_pool(name="p0strip", bufs=1)  # one buf per tag (2 tags)
            p0strip_pool = p0s_cm.__enter__()
            p0c_cm = tc.tile_pool(name="p0c", bufs=1)
            p0c = p0c_cm.__enter__()
            cos_own_sb = p0c.tile([64, 2, CH], F32, tag="coso")
            sin_own_sb = p0c.tile([64, 2, CH], F32, tag="sino")
            nc.sync.dma_start(out=cos_own_sb[:], in_=cos_own[:])
            nc.sync.dma_start(out=sin_own_sb[:], in_=sin_own[:])
            qbT_e = p0c.tile([64, QH], F32, tag="qbte")
            qbT_o = p0c.tile([64, QH], F32, tag="qbto")
            nc.sync.dma_start(out=qbT_e[:], in_=qbT[0:64, :])
            nc.sync.dma_start(out=qbT_o[:], in_=qbT[64:P, :])

            p0_strips = {}
            for _sl in range(2):
                _st = p0strip_pool.tile(
                    [P, ND, CH], BF16, tag=f"p0strip{_sl}", name=f"ostrip{_sl}"
                )
                nc.sync.dma_start(out=_st[:], in_=own[_sl])
                p0_strips[_sl] = _st

            def p0_heads(sl, h0, h1, group, qrot_eng=None):
                """Emit Q-proj for heads [h0,h1) of slot sl; yield after each
                `group` matmuls. Weight loads alternate Pool/SP queues."""
                qrot_eng = qrot_eng or nc.sync
                st = p0_strips[sl]
                for h in range(h0, h1):
                    w = wpool.tile([P, ND, P], BF16, tag="w", name=f"qw{sl}_{h}")
                    eng = (nc.gpsimd, nc.sync)[h % 2]
                    eng.dma_start(out=w[:], in_=qwT[h])
                    ps = pspool.tile([P, CH], F32, tag="ps", name="ps_q")
                    for dtg in range(ND // group):
                        for k in range(group):
                            dt = dtg * group + k
                            nc.tensor.matmul(
                                ps[:], lhsT=w[:, dt, :], rhs=st[:, dt, :],
                                start=(dt == 0), stop=(dt == ND - 1),
                            )
                        yield
                    qrot = evpool.tile([P, CH], BF16, tag="ev", name="qrot")
                    rotary_evict(
                        ps, qrot,
                        cos_own_sb[:, sl, :], sin_own_sb[:, sl, :],
                        qbT_e[:, h : h + 1], qbT_o[:, h : h + 1],
                    )
                    qrot_eng.dma_start(out=qT_i[sl, h], in_=qrot[:])

            for _ in p0_heads(0, 0, QH, ND):
                pass
            for _ in p0_heads(1, 0, 8, ND):
                pass

            def p4_half(hf, yield_every, otr, wb4pool, ob_sb):
                """Emit o-proj for token-slot half `hf` (ttiles 4hf..4hf+3)."""
                for hq in range(0, QH, 8):
                    nc.sync.dma_start(
                        out=otr[:, hq : hq + 8, :],
                        in_=oT_i[hf, hq : hq + 8].rearrange("h p t -> p h t"),
                    )
                for e in range(8):
                    ps4 = [
                        pspool.tile([P, 512], F32, tag="ps", name=f"ps4_{i}")
                        for i in range(4)
                    ]
                    cnt = 0
                    for f4 in range(ND // 4):
                        ow = wb4pool.tile([P, 4, 512], BF16, tag="wb4", name="ow")
                        nc.gpsimd.dma_start(
                            out=ow[:],
                            in_=owT[e, 4 * f4 : 4 * f4 + 4].rearrange("d p j -> p d j"),
                        )
                        for df in range(4):
                            ft = 4 * f4 + df
                            for tsub in range(4):
                                nc.tensor.matmul(
                                    ps4[tsub][:],
                                    lhsT=otr[:, ft, tsub * P : (tsub + 1) * P],
                                    rhs=ow[:, df, :],
                                    start=(ft == 0), stop=(ft == ND - 1),
                                )
                                cnt += 1
                                if cnt % yield_every == 0:
                                    yield
                    for tsub in range(4):
                        osb = evpool.tile([P, 512], F32, tag="ev4", name="osb4")
                        nc.vector.tensor_add(
                            osb[:], ps4[tsub][:], ob_sb[:, e * 512 : (e + 1) * 512]
                        )
                        nc.sync.dma_start(
                            out=out[hf * 4 + tsub, :, e * 512 : (e + 1) * 512],
                            in_=osb[:],
                        )

            def attn_slot(sl, feeder):
                n_kb = NKB[sl]
                nrow = n_kb // 8  # kT rows per head ([P,1024] each)
                with (
                    tc.tile_pool(name=f"mask{sl}", bufs=1) as mpool,
                    tc.tile_pool(name=f"v4{sl}", bufs=1) as v4pool,
                ):
                    msk = mpool.tile([P, 8, CH], BF16, tag="mask", name="msk")
                    nc.scalar.dma_start(
                        out=msk[:], in_=masks[sl].rearrange("m k q -> k m q")
                    )
                    # merged v for ALL 8 kv heads, whole slot: [keys, kb, 8*128]
                    # The slot-0 loads are scheduling-floored past the
                    # collective's completion so their collective-wait never
                    # stalls a DMA ring (whose completion counters gate
                    # unrelated queues via shared semaphore recycling).
                    v4 = v4pool.tile([P, n_kb, 1024], BF16, tag="v4", name="v4")
                    ld = nc.scalar if sl == 0 else nc.gpsimd
                    with tc.tile_wait_until(0.53, enable=(sl == 0)):
                        # merged kT for ALL 8 kv heads, whole slot, one DMA:
                        # slot 0 needs only the even rows (first 1024 keys).
                        ktall = kvpool.tile([P, 8 * nrow, 1024], BF16, tag="kt", name="ktall")
                        ld.dma_start(
                            out=ktall[:],
                            in_=cc_m[0, 0 : 8 * nrow].rearrange("r p t -> p r t"),
                        )
                        ld.dma_start(
                            out=v4[:, 0:n_kb, :],
                            in_=cc_m[1, 0:n_kb].rearrange("b p j -> p b j"),
                        )
                    for kv in range(KVH):
                        qt4 = qtpool.tile([P, 4, CH], BF16, tag="qt", name="qt4")
                        nc.sync.dma_start(
                            out=qt4[:],
                            in_=qT_i[sl, kv :: KVH].rearrange("g p t -> p g t"),
                        )
                        for g in range(4):
                            h = kv + KVH * g
                            oT_ps = pspool.tile([P, CH], F32, tag="ps", name="oT_ps")
                            sums_ps = pspool.tile([P, CH], F32, tag="ps", name="sums_ps")
                            for kb in range(n_kb):
                                st_ps = pspool.tile([P, CH], F32, tag="ps", name="st_ps")
                                nc.tensor.matmul(
                                    st_ps[:],
                                    lhsT=ktall[:, (kb // 4) * LKV + kv % LKV,
                                               (kv // LKV) * 512
                                               + (kb % 4) * P : (kv // LKV) * 512
                                               + (kb % 4 + 1) * P],
                                    rhs=qt4[:, g, :], start=True, stop=True,
                                )
                                pt = ptpool.tile([P, CH], BF16, tag="pt", name="pt")
                                nc.scalar.activation(pt[:], st_ps[:], EXP, scale=SCALE)
                                if sl == 0 or kb >= 8:
                                    mi = kb if sl == 0 else kb - 8
                                    nc.vector.tensor_mul(pt[:], pt[:], msk[:, mi, :])
                                if kb % 2 == 0:
                                    pt_prev = pt
                                else:
                                    pp = ptpool.tile([P, CH], BF16, tag="ptp", name="pp")
                                    nc.vector.tensor_add(pp[:], pt_prev[:], pt[:])
                                    if kb % 4 == 1:
                                        pp_prev = pp
                                    else:
                                        pq = ptpool.tile([P, CH], BF16, tag="ptq", name="pq")
                                        nc.vector.tensor_add(pq[:], pp_prev[:], pp[:])
                                        # slot-0 feeder has surplus yields:
                                        # pull extra fill so PE doesn't wait
                                        # on the DVE chain feeding pq
                                        if feeder is not None and sl == 0:
                                            for _ in range(2):
                                                next(feeder, None)
                                        nc.tensor.matmul(
                                            sums_ps[:], lhsT=ones_mat[:], rhs=pq[:],
                                            start=(kb == 3), stop=(kb == n_kb - 1),
                                        )
                                nc.tensor.matmul(
                                    oT_ps[:],
                                    lhsT=v4[:, kb, kv * P : (kv + 1) * P],
                                    rhs=pt[:],
                                    start=(kb == 0), stop=(kb == n_kb - 1),
                                )
                                if feeder is not None:
                                    next(feeder, None)
                            rb = ptpool.tile([P, CH], F32R, tag="ptr", name="rb")
                            with nc.allow_low_precision(reason="f32r softmax denom"):
                                nc.vector.reciprocal(rb[:], sums_ps[:])
                            osb = evpool.tile([P, CH], BF16, tag="evb", name="osb")
                            nc.vector.tensor_mul(osb[:], oT_ps[:], rb[:])
                            nc.sync.dma_start(out=oT_i[sl, h], in_=osb[:])
                    if feeder is not None:
                        for _ in feeder:
                            pass

            # ==== P3 slot 0 woven with the tail of P0 slot 1 ====
            attn_slot(0, p0_heads(1, 8, QH, 2, qrot_eng=nc.sync))
            p0c_cm.__exit__(None, None, None)
            p0s_cm.__exit__(None, None, None)
            qw_cm.__exit__(None, None, None)

            # ==== P3 slot 1 woven with P4 half 0; then P4 half 1 ====
            with tc.tile_pool(name="obp", bufs=1) as obp:
                ob_sb = obp.tile([P, D], F32, tag="ob")
                nc.sync.dma_start(
                    out=ob_sb[:], in_=ob.ap()[None, :].partition_broadcast(P)
                )
                with (
                    tc.tile_pool(name="p4a", bufs=1) as p4a,
                    tc.tile_pool(name="wb4a", bufs=2) as wb4a,
                ):
                    otr0 = p4a.tile([P, QH, CH], BF16, tag="ot0")
                    attn_slot(1, p4_half(0, 2, otr0, wb4a, ob_sb))
                with (
                    tc.tile_pool(name="p4b", bufs=1) as p4b,
                    tc.tile_pool(name="wb4b", bufs=3) as wb4b,
                ):
                    otr1 = p4b.tile([P, QH, CH], BF16, tag="ot1")
                    for _ in p4_half(1, 1 << 30, otr1, wb4b, ob_sb):
                        pass
            r_cm.__exit__(None, None, None)
            pt_cm.__exit__(None, None, None)
            qt_cm.__exit__(None, None, None)
            kv_cm.__exit__(None, None, None)

    nc.compile()
    return nc


def _get_nc():
    if "nc" not in _CACHE:
        _CACHE["nc"] = _build()
    return _CACHE["nc"]


_PERM = np.concatenate([np.arange(0, P, 2), np.arange(1, P, 2)])


def _prep_shared(qw_w, qw_b, kw_w, kw_b, vw_w, vw_b, ow_w, ow_b, fc, fs):
    f32 = np.float32
    c = np.ascontiguousarray
    # [h, dp, dt, fp] = w[h*128 + perm[fp], dt*128 + dp]
    qq = qw_w.reshape(QH, P, D)[:, _PERM, :]                      # [h, fp, d]
    qwT = c(qq.reshape(QH, P, ND, P).transpose(0, 3, 2, 1).astype(BF))
    kk = kw_w.reshape(KVH, P, D)[:, _PERM, :]
    kwT = c(kk.reshape(KVH, P, ND, P).transpose(0, 3, 2, 1).astype(BF))
    # per 4-head half: [dt, dp, j] = vw[hs*512 + j, dt*128 + dp]
    vwT = c(vw_w.reshape(2, 512, ND, P).transpose(0, 2, 3, 1).astype(BF))
    # [es, ft, fp, j] = ow[es*512 + j, ft*128 + fp]
    owT = c(ow_w.reshape(8, 512, ND, P).transpose(0, 2, 3, 1).astype(BF))
    cos_all = c(fc.T.astype(f32))  # [64, S]
    sin_all = c(fs.T.astype(f32))
    qbT = c(qw_b.reshape(QH, P)[:, _PERM].T.astype(f32))
    kbT = c(kw_b.reshape(KVH, P)[:, _PERM].T.astype(f32))
    return dict(
        qwT=qwT, kwT=kwT, vwT=vwT, owT=owT,
        cos_all=cos_all, sin_all=sin_all, qbT=qbT, kbT=kbT,
        vb=c(vw_b.astype(f32)), ob=c(ow_b.astype(f32)),
    )


def _masks_for(chunks):
    m = np.zeros((2, 8, P, CH), BF)
    kp = np.arange(P)[:, None]
    qi = np.arange(CH)[None, :]
    for sl in range(2):
        q0 = chunks[sl] * CH
        for mi in range(8):
            kb = mi if sl == 0 else mi + 8
            m[sl, mi] = (kb * P + kp <= q0 + qi).astype(BF)
    return m


def _core_chunks(core):
    b, par = core // 2, core % 2
    return b, ((0, 3) if par == 0 else (1, 2))


def _make_in_maps(inputs):
    """inputs: dict with the reference's setup_inputs() keys (numpy)."""
    g = lambda k: np.asarray(inputs[k])
    shared = _prep_shared(
        g("qw_w"), g("qw_b"), g("kw_w"), g("kw_b"), g("vw_w"), g("vw_b"),
        g("ow_w"), g("ow_b"), g("freqs_cos"), g("freqs_sin"),
    )
    input = g("input")
    in_maps = []
    for core in range(NCORES):
        b, chunks = _core_chunks(core)
        r = core % 2
        x = input[b].astype(np.float32)  # [S, D]
        # [s, dp, dt, t] = x[s*512 + t, dt*128 + dp]
        strips = np.ascontiguousarray(
            x.reshape(NCH, CH, ND, P).transpose(0, 3, 2, 1).astype(BF)
        )
        own = np.ascontiguousarray(strips[list(chunks)])
        cos_own = np.ascontiguousarray(
            np.stack([shared["cos_all"][:, c * CH : (c + 1) * CH] for c in chunks], 1)
        )
        sin_own = np.ascontiguousarray(
            np.stack([shared["sin_all"][:, c * CH : (c + 1) * CH] for c in chunks], 1)
        )
        vb_loc = shared["vb"][r * 512 : (r + 1) * 512]
        vbm = np.concatenate([
            vb_loc * (1.0 if r == 0 else 0.0),
            vb_loc * (1.0 if r == 1 else 0.0),
        ]).astype(np.float32)
        mm = np.zeros((P, 2), np.float32)
        mm[:, r] = 1.0
        m = dict(
            qwT=shared["qwT"], owT=shared["owT"],
            cos_all=shared["cos_all"], sin_all=shared["sin_all"],
            qbT=shared["qbT"], ob=shared["ob"],
            kwT_loc=np.ascontiguousarray(shared["kwT"][r * LKV : (r + 1) * LKV]),
            vwT_loc=np.ascontiguousarray(shared["vwT"][r]),
            kbT_loc=np.ascontiguousarray(shared["kbT"][:, r * LKV : (r + 1) * LKV]),
            vbm=vbm, mm=mm,
            ones=np.ones((P, P), BF),
            own_strips=own, in_strips=strips,
            cos_own=cos_own, sin_own=sin_own, masks=_masks_for(chunks),
        )
        in_maps.append(m)
    return in_maps


def kernel(input, freqs_cos, freqs_sin, qw_w, qw_b, kw_w, kw_b, vw_w, vw_b,
           ow_w, ow_b, start_pos):
    in_maps = _make_in_maps(dict(
        input=input, freqs_cos=freqs_cos, freqs_sin=freqs_sin,
        qw_w=qw_w, qw_b=qw_b, kw_w=kw_w, kw_b=kw_b, vw_w=vw_w, vw_b=vw_b,
        ow_w=ow_w, ow_b=ow_b,
    ))
    nc = _get_nc()
    res = run_bass_kernel_spmd(nc, in_maps, list(range(NCORES)))

    out = np.empty((B, S, D), np.float32)
    for core in range(NCORES):
        b, chunks = _core_chunks(core)
        r = res.results[core]["out"].reshape(2, CH, D)
        for sl in range(2):
            c0 = chunks[sl] * CH
            out[b, c0 : c0 + CH, :] = r[sl]
    return out


# revision 51
# speedup vs baseline: 1.0094x; 1.0094x over previous
"""GQA attention prefill (B=4, S=2048, D=4096, 32 q-heads / 8 kv-heads, rotary,
causal) on 8 TRN2 NeuronCores.

Sharding: token-parallel attention + tensor-parallel K/V projection.

Token side: core c handles batch c//2 and two 512-token chunks of its
sequence: chunks {0,3} for even cores, {1,2} for odd cores (zigzag splits the
causal triangle evenly). Each core computes the full Q projection for its
tokens, attention for all 32 heads over its tokens, and the output projection
for its tokens. Outputs are disjoint token slices, gathered on host.

K/V side: the 8 kv heads are tensor-parallel across the core PAIR that shares
a batch — role r = c%2 computes only kv heads {4r..4r+3} over the full
2048-token prefix (halves the K/V projection PE work). Each K/V tile is
written twice into a GLOBAL-layout DRAM buffer — once at the role-0 position
scaled by m0, once at the role-1 position scaled by m1, where (m0,m1) are
per-core {0,1} input scalars — so every core produces the same-layout buffer
with zeros in the partner's slots. The buffer is duplicated into two halves
(one DRAM->DRAM copy) and a pairwise ReduceScatter(add) (replica groups
[[0,1],[2,3],[4,5],[6,7]]) hands BOTH cores the fully merged global K/V at
the same address: zero+x = x, and both scatter halves are identical. The
SPMD program stays identical on all cores — no role-dependent addresses.

The collective fires right after the K/V pass and flies during the Q
projection (~380us of PE work), so its ~225us latency is hidden.

DMA-queue routing (each queue's sequencer serializes its DMAs through the
transfer, so dispatch is a per-queue resource):
  - sync (SP):    strips, odd q/v weights, v4/qt4/mask loads, oT/out writes
  - gpsimd (Pool): even q/v weights, kw, ow streams, the collective
  - scalar (Act):  K/V global-buffer writes, the D2D duplicate, qrot writes
                   (Act is otherwise idle outside attention)
  - vector (DVE):  kT loads for attention

Precision: projections bf16 x bf16 -> f32 PSUM; rotary applied on PSUM f32,
K/Q written back as bf16; softmax denominator via ones-vector matmul in PSUM
f32; AV and the output projection bf16.

Layout conventions:
  - activations for QK^T are kept transposed: [head_dim (partitions), tokens]
  - rotary pairs are de-interleaved (even dims -> partitions 0-63, odd ->
    64-127) via a host-side permutation of the qw/kw rows.
  - attention runs scores-transposed: ST[key, query] = kT.T @ qT, softmax over
    the partition (key) axis, denominator via ones-vector matmul, no
    max-subtraction (scores are O(1)).
"""

import numpy as np
import ml_dtypes

import concourse.bacc as bacc
import concourse.bass as bass
import concourse.tile as tile
from concourse import library_config, mybir
from concourse.bass_utils import run_bass_kernel_spmd

F32 = mybir.dt.float32
F32R = mybir.dt.float32r
BF16 = mybir.dt.bfloat16
EXP = mybir.ActivationFunctionType.Exp
ADD = mybir.AluOpType.add
MULT = mybir.AluOpType.mult

B, S, D = 4, 2048, 4096
QH, KVH, HEAD = 32, 8, 128
P = 128
CH = 512                # token chunk (= query tile)
NCH = S // CH           # 4 chunks per sequence
ND = D // P             # 32 d-tiles
LKV = 4                 # kv heads per core (tensor-parallel within pair)
NCORES = 8
NKB = (8, 16)           # key-blocks per query slot (padded, uniform)
SCALE = 1.0 / np.sqrt(HEAD)
BF = ml_dtypes.bfloat16
PAIRS = [[0, 1], [2, 3], [4, 5], [6, 7]]

_CACHE = {}


def _build():
    nc = bacc.Bacc("TRN2", target_bir_lowering=False, debug=False, num_devices=NCORES)

    # ---- per-core external inputs ----
    own = nc.dram_tensor("own_strips", [2, P, ND, CH], BF16, kind="ExternalInput")
    pref = nc.dram_tensor("in_strips", [NCH, P, ND, CH], BF16, kind="ExternalInput")
    qwT = nc.dram_tensor("qwT", [QH, P, ND, P], BF16, kind="ExternalInput")
    kwT = nc.dram_tensor("kwT_loc", [LKV, P, ND, P], BF16, kind="ExternalInput")
    vwT = nc.dram_tensor("vwT_loc", [ND, P, LKV * HEAD], BF16, kind="ExternalInput")
    owT = nc.dram_tensor("owT", [8, ND, P, 512], BF16, kind="ExternalInput")
    cos_own = nc.dram_tensor("cos_own", [64, 2, CH], F32, kind="ExternalInput")
    sin_own = nc.dram_tensor("sin_own", [64, 2, CH], F32, kind="ExternalInput")
    cos_all = nc.dram_tensor("cos_all", [64, S], F32, kind="ExternalInput")
    sin_all = nc.dram_tensor("sin_all", [64, S], F32, kind="ExternalInput")
    qbT = nc.dram_tensor("qbT", [P, QH], F32, kind="ExternalInput")
    kbT = nc.dram_tensor("kbT_loc", [P, LKV], F32, kind="ExternalInput")
    vbm = nc.dram_tensor("vbm", [2 * LKV * HEAD], F32, kind="ExternalInput")
    ob = nc.dram_tensor("ob", [D], F32, kind="ExternalInput")
    masks = nc.dram_tensor("masks", [2, 8, P, CH], BF16, kind="ExternalInput")
    ones = nc.dram_tensor("ones", [P, P], BF16, kind="ExternalInput")
    mm = nc.dram_tensor("mm", [P, 2], F32, kind="ExternalInput")

    # ---- internal DRAM ----
    # Global K/V layout (kind, row, P, 1024):
    #   kind 0 = kT: kv head j (GLOBAL), chunk tg -> row 2j + tg//2,
    #       cols (tg%2)*512.  (kT_g[j] = [128 hd, 2048 keys] as 2 rows)
    #   kind 1 = v: key-block kb -> row kb, cols j*128+hd (j GLOBAL kv head)
    # cc_in has two identical halves of this layout (masked: partner slots
    # zero); ReduceScatter(add) over the pair yields the merged layout cc_m
    # on both cores.
    cc_in = nc.dram_tensor("cc_in", [2, 2, 2 * KVH, P, 1024], BF16)
    cc_m = nc.dram_tensor("cc_m", [2, 2 * KVH, P, 1024], BF16)
    qT_i = nc.dram_tensor("qT_i", [2, QH, P, CH], BF16)
    oT_i = nc.dram_tensor("oT_i", [2, QH, P, CH], BF16)

    out = nc.dram_tensor("out", [8, P, D], F32, kind="ExternalOutput")

    with tile.TileContext(nc) as tc:
        nc.gpsimd.load_library(library_config.lib)
        with (
            tc.tile_pool(name="const", bufs=1) as const,
            tc.tile_pool(name="ev", bufs=3) as evpool,
            tc.tile_pool(name="rt", bufs=4) as rtpool,
            tc.tile_pool(name="ps", bufs=8, space="PSUM") as pspool,
        ):
            kbT_e = const.tile([64, LKV], F32, tag="kbte")
            kbT_o = const.tile([64, LKV], F32, tag="kbto")
            nc.scalar.dma_start(out=kbT_e[:], in_=kbT[0:64, :])
            nc.scalar.dma_start(out=kbT_o[:], in_=kbT[64:P, :])
            # all-ones stationary matrix: the denominator matmul writes the
            # key-sum replicated across ALL partitions, so the softmax tail
            # needs no partition_broadcast (no Pool-ring round trip).
            ones_mat = const.tile([P, P], BF16, tag="oc")
            nc.scalar.dma_start(out=ones_mat[:], in_=ones[:])
            mm_sb = const.tile([P, 2], F32, tag="mm")
            nc.scalar.dma_start(out=mm_sb[:], in_=mm[:])

            def rotary_evict(ps, dst, cos_ap, sin_ap, be, bo):
                """dst[0:64]=(pe+be)*cos-(po+bo)*sin; dst[64:128]=(pe+be)*sin+(po+bo)*cos"""
                pe, po = ps[0:64, :], ps[64:128, :]
                t1 = rtpool.tile([64, CH], F32, tag="rt", name="t1")
                t2 = rtpool.tile([64, CH], F32, tag="rt", name="t2")
                nc.vector.scalar_tensor_tensor(t1[:], pe, be, cos_ap, ADD, MULT)
                nc.vector.scalar_tensor_tensor(t2[:], po, bo, sin_ap, ADD, MULT)
                nc.vector.tensor_sub(dst[0:64, :], t1[:], t2[:])
                t3 = rtpool.tile([64, CH], F32, tag="rt", name="t3")
                t4 = rtpool.tile([64, CH], F32, tag="rt", name="t4")
                nc.vector.scalar_tensor_tensor(t3[:], pe, be, sin_ap, ADD, MULT)
                nc.vector.scalar_tensor_tensor(t4[:], po, bo, cos_ap, ADD, MULT)
                nc.vector.tensor_add(dst[64:128, :], t3[:], t4[:])

            p01_cm = tc.tile_pool(name="strip", bufs=3)
            strip_pool = p01_cm.__enter__()
            kw_cm = tc.tile_pool(name="kw", bufs=1)
            kwpool = kw_cm.__enter__()
            kw_tiles = {}

            # ============ P1: K/V projection, LOCAL kv heads only ============
            with tc.tile_pool(name="p1c", bufs=1) as p1c:
                cos_all_sb = p1c.tile([64, S], F32, tag="cosa")
                sin_all_sb = p1c.tile([64, S], F32, tag="sina")
                nc.sync.dma_start(out=cos_all_sb[:], in_=cos_all[:])
                nc.sync.dma_start(out=sin_all_sb[:], in_=sin_all[:])
                vbm_sb = p1c.tile([P, 2 * LKV * HEAD], F32, tag="vbm")
                nc.sync.dma_start(
                    out=vbm_sb[:], in_=vbm.ap()[None, :].partition_broadcast(P)
                )
                with tc.tile_pool(name="wb", bufs=6) as wbpool, \
                     tc.tile_pool(name="stg", bufs=1) as stgpool:
                    for pr in range(2):
                        strips = []
                        for i in range(2):
                            st = strip_pool.tile(
                                [P, ND, CH], BF16, tag="strip", name=f"strip{pr}_{i}"
                            )
                            # pr0's second strip rides the Act ring's idle
                            # start so both strips are up before the K pass;
                            # pr0's first strip loads in two halves so the
                            # first matmul can start after half the transfer
                            if pr == 0 and i == 0:
                                nc.sync.dma_start(
                                    out=st[:, 0 : ND // 2, :],
                                    in_=pref[0, :, 0 : ND // 2, :],
                                )
                                nc.sync.dma_start(
                                    out=st[:, ND // 2 :, :],
                                    in_=pref[0, :, ND // 2 :, :],
                                )
                            else:
                                eng = nc.scalar if (pr == 0 and i == 1) else nc.sync
                                eng.dma_start(out=st[:], in_=pref[2 * pr + i])
                            strips.append(st)
                        # K-pass (weight-stationary, out = kT [f, t]), 4 local
                        # heads. ts outer / kv inner: the 4 heads of one chunk
                        # land in consecutive global rows tg*4..tg*4+3, staged
                        # in SBUF and written per chunk as ONE 4-row DMA per
                        # scatter half (8x fewer ring-dispatch serializations).
                        for ts in range(2):
                            tg = 2 * pr + ts
                            ksg = stgpool.tile([P, LKV, 2, CH], BF16, tag="ksg",
                                               name="ksg")
                            for kv in range(LKV):
                                if kv in kw_tiles:
                                    w = kw_tiles[kv]
                                else:
                                    w = kwpool.tile([P, ND, P], BF16, tag=f"kw{kv}",
                                                    name=f"kw{kv}")
                                    nc.gpsimd.dma_start(out=w[:], in_=kwT[kv])
                                    kw_tiles[kv] = w
                                ps = pspool.tile([P, CH], F32, tag="ps", name="ps_k")
                                for dt in range(ND):
                                    nc.tensor.matmul(
                                        ps[:], lhsT=w[:, dt, :], rhs=strips[ts][:, dt, :],
                                        start=(dt == 0), stop=(dt == ND - 1),
                                    )
                                krot = evpool.tile([P, CH], F32, tag="ev", name="krot")
                                rotary_evict(
                                    ps, krot,
                                    cos_all_sb[:, tg * CH : (tg + 1) * CH],
                                    sin_all_sb[:, tg * CH : (tg + 1) * CH],
                                    kbT_e[:, kv : kv + 1], kbT_o[:, kv : kv + 1],
                                )
                                for q in range(2):
                                    nc.vector.tensor_scalar_mul(
                                        ksg[:, kv, q, :], krot[:], mm_sb[:, q : q + 1]
                                    )
                            for half, eng in ((0, nc.scalar), (1, nc.gpsimd)):
                                eng.dma_start(
                                    out=cc_in[half, 0, tg * LKV : (tg + 1) * LKV]
                                    .rearrange("r p t -> p r t"),
                                    in_=ksg[:],
                                )
                        # V-pass (input-stationary, out = v [t, hd]), 4 local heads
                        psv = [
                            pspool.tile([P, 512], F32, tag="ps", name=f"psv{i}")
                            for i in range(8)
                        ]
                        for dt in range(ND):
                            vw = wbpool.tile([P, 512], BF16, tag="wb", name="vw")
                            eng = nc.gpsimd if dt % 2 == 0 else nc.sync
                            eng.dma_start(out=vw[:], in_=vwT[dt])
                            for ts in range(2):
                                for tt in range(4):
                                    nc.tensor.matmul(
                                        psv[ts * 4 + tt][:],
                                        lhsT=strips[ts][:, dt, tt * P : (tt + 1) * P],
                                        rhs=vw[:],
                                        start=(dt == 0), stop=(dt == ND - 1),
                                    )
                        for ts in range(2):
                            kb0 = (2 * pr + ts) * 4
                            vsg = stgpool.tile([P, 4, 1024], BF16, tag="vsg", name="vsg")
                            for tt in range(4):
                                # masked double-write into both role column
                                # halves of the staged global v rows
                                for q in range(2):
                                    nc.vector.scalar_tensor_tensor(
                                        vsg[:, tt, q * 512 : (q + 1) * 512],
                                        psv[ts * 4 + tt][:],
                                        mm_sb[:, q : q + 1],
                                        vbm_sb[:, q * 512 : (q + 1) * 512],
                                        MULT, ADD,
                                    )
                            for half, eng in ((0, nc.scalar), (1, nc.gpsimd)):
                                eng.dma_start(
                                    out=cc_in[half, 1, kb0 : kb0 + 4]
                                    .rearrange("r p t -> p r t"),
                                    in_=vsg[:],
                                )

            kw_cm.__exit__(None, None, None)
            p01_cm.__exit__(None, None, None)

            # Pairwise ReduceScatter(add): both scatter halves are written
            # identically by the evictions, so every core receives the merged
            # global K/V at cc_m. Emitted before any Q-projection instruction
            # so the scheduler places it at the head of the Pool ring.
            nc.gpsimd.collective_compute(
                "ReduceScatter",
                mybir.AluOpType.add,
                replica_groups=PAIRS,
                ins=[cc_in.ap()],
                outs=[cc_m.ap()],
            )

            kv_cm = tc.tile_pool(name="kvS", bufs=1)
            kvpool = kv_cm.__enter__()
            qt_cm = tc.tile_pool(name="qtS", bufs=2)
            qtpool = qt_cm.__enter__()
            pt_cm = tc.tile_pool(name="ptS", bufs=4)
            ptpool = pt_cm.__enter__()
            r_cm = tc.tile_pool(name="rS", bufs=2)
            rpool = r_cm.__enter__()
            qw_cm = tc.tile_pool(name="qw", bufs=3)
            wpool = qw_cm.__enter__()

            # ============ P0: Q projection + rotary -> qT_i (bf16) ============
            # Slot 0 + 24 heads of slot 1 emitted eagerly (covers the
            # collective); the last 8 heads are woven into attention slot 0.
            p0s_cm = tc.tile_pool(name="p0strip", bufs=1)  # one buf per tag (2 tags)
            p0strip_pool = p0s_cm.__enter__()
            p0c_cm = tc.tile_pool(name="p0c", bufs=1)
            p0c = p0c_cm.__enter__()
            cos_own_sb = p0c.tile([64, 2, CH], F32, tag="coso")
            sin_own_sb = p0c.tile([64, 2, CH], F32, tag="sino")
            nc.sync.dma_start(out=cos_own_sb[:], in_=cos_own[:])
            nc.sync.dma_start(out=sin_own_sb[:], in_=sin_own[:])
            qbT_e = p0c.tile([64, QH], F32, tag="qbte")
            qbT_o = p0c.tile([64, QH], F32, tag="qbto")
            nc.sync.dma_start(out=qbT_e[:], in_=qbT[0:64, :])
            nc.sync.dma_start(out=qbT_o[:], in_=qbT[64:P, :])

            p0_strips = {}
            for _sl in range(2):
                _st = p0strip_pool.tile(
                    [P, ND, CH], BF16, tag=f"p0strip{_sl}", name=f"ostrip{_sl}"
                )
                nc.sync.dma_start(out=_st[:], in_=own[_sl])
                p0_strips[_sl] = _st

            def p0_heads(sl, h0, h1, group, qrot_eng=None):
                """Emit Q-proj for heads [h0,h1) of slot sl; yield after each
                `group` matmuls. Weight loads alternate Pool/SP queues."""
                qrot_eng = qrot_eng or nc.sync
                st = p0_strips[sl]
                for h in range(h0, h1):
                    w = wpool.tile([P, ND, P], BF16, tag="w", name=f"qw{sl}_{h}")
                    eng = (nc.gpsimd, nc.sync)[h % 2]
                    eng.dma_start(out=w[:], in_=qwT[h])
                    ps = pspool.tile([P, CH], F32, tag="ps", name="ps_q")
                    for dtg in range(ND // group):
                        for k in range(group):
                            dt = dtg * group + k
                            nc.tensor.matmul(
                                ps[:], lhsT=w[:, dt, :], rhs=st[:, dt, :],
                                start=(dt == 0), stop=(dt == ND - 1),
                            )
                        yield
                    qrot = evpool.tile([P, CH], BF16, tag="ev", name="qrot")
                    rotary_evict(
                        ps, qrot,
                        cos_own_sb[:, sl, :], sin_own_sb[:, sl, :],
                        qbT_e[:, h : h + 1], qbT_o[:, h : h + 1],
                    )
                    qrot_eng.dma_start(out=qT_i[sl, h], in_=qrot[:])

            for _ in p0_heads(0, 0, QH, ND):
                pass
            for _ in p0_heads(1, 0, 8, ND):
                pass

            def p4_half(hf, yield_every, otr, wb4pool, ob_sb):
                """Emit o-proj for token-slot half `hf` (ttiles 4hf..4hf+3)."""
                for hq in range(0, QH, 8):
                    nc.sync.dma_start(
                        out=otr[:, hq : hq + 8, :],
                        in_=oT_i[hf, hq : hq + 8].rearrange("h p t -> p h t"),
                    )
                for e in range(8):
                    ps4 = [
                        pspool.tile([P, 512], F32, tag="ps", name=f"ps4_{i}")
                        for i in range(4)
                    ]
                    cnt = 0
                    for f4 in range(ND // 4):
                        ow = wb4pool.tile([P, 4, 512], BF16, tag="wb4", name="ow")
                        nc.gpsimd.dma_start(
                            out=ow[:],
                            in_=owT[e, 4 * f4 : 4 * f4 + 4].rearrange("d p j -> p d j"),
                        )
                        for df in range(4):
                            ft = 4 * f4 + df
                            for tsub in range(4):
                                nc.tensor.matmul(
                                    ps4[tsub][:],
                                    lhsT=otr[:, ft, tsub * P : (tsub + 1) * P],
                                    rhs=ow[:, df, :],
                                    start=(ft == 0), stop=(ft == ND - 1),
                                )
                                cnt += 1
                                if cnt % yield_every == 0:
                                    yield
                    for tsub in range(4):
                        osb = evpool.tile([P, 512], F32, tag="ev4", name="osb4")
                        nc.vector.tensor_add(
                            osb[:], ps4[tsub][:], ob_sb[:, e * 512 : (e + 1) * 512]
                        )
                        nc.sync.dma_start(
                            out=out[hf * 4 + tsub, :, e * 512 : (e + 1) * 512],
                            in_=osb[:],
                        )

            def attn_slot(sl, feeder):
                n_kb = NKB[sl]
                nrow = n_kb // 8  # kT rows per head ([P,1024] each)
                with (
                    tc.tile_pool(name=f"mask{sl}", bufs=1) as mpool,
                    tc.tile_pool(name=f"v4{sl}", bufs=1) as v4pool,
                ):
                    msk = mpool.tile([P, 8, CH], BF16, tag="mask", name="msk")
                    nc.scalar.dma_start(
                        out=msk[:], in_=masks[sl].rearrange("m k q -> k m q")
                    )
                    # merged v for ALL 8 kv heads, whole slot: [keys, kb, 8*128]
                    # The slot-0 loads are scheduling-floored past the
                    # collective's completion so their collective-wait never
                    # stalls a DMA ring (whose completion counters gate
                    # unrelated queues via shared semaphore recycling).
                    v4 = v4pool.tile([P, n_kb, 1024], BF16, tag="v4", name="v4")
                    ld = nc.scalar if sl == 0 else nc.gpsimd
                    with tc.tile_wait_until(0.53, enable=(sl == 0)):
                        # merged kT for ALL 8 kv heads, whole slot, one DMA:
                        # slot 0 needs only the even rows (first 1024 keys).
                        ktall = kvpool.tile([P, 8 * nrow, 1024], BF16, tag="kt", name="ktall")
                        ld.dma_start(
                            out=ktall[:],
                            in_=cc_m[0, 0 : 8 * nrow].rearrange("r p t -> p r t"),
                        )
                        ld.dma_start(
                            out=v4[:, 0:n_kb, :],
                            in_=cc_m[1, 0:n_kb].rearrange("b p j -> p b j"),
                        )
                    for kv in range(KVH):
                        qt4 = qtpool.tile([P, 4, CH], BF16, tag="qt", name="qt4")
                        nc.sync.dma_start(
                            out=qt4[:],
                            in_=qT_i[sl, kv :: KVH].rearrange("g p t -> p g t"),
                        )
                        for g in range(4):
                            h = kv + KVH * g
                            oT_ps = pspool.tile([P, CH], F32, tag="ps", name="oT_ps")
                            sums_ps = pspool.tile([P, CH], F32, tag="ps", name="sums_ps")
                            for kb in range(n_kb):
                                st_ps = pspool.tile([P, CH], F32, tag="ps", name="st_ps")
                                nc.tensor.matmul(
                                    st_ps[:],
                                    lhsT=ktall[:, (kb // 4) * LKV + kv % LKV,
                                               (kv // LKV) * 512
                                               + (kb % 4) * P : (kv // LKV) * 512
                                               + (kb % 4 + 1) * P],
                                    rhs=qt4[:, g, :], start=True, stop=True,
                                )
                                pt = ptpool.tile([P, CH], BF16, tag="pt", name="pt")
                                nc.scalar.activation(pt[:], st_ps[:], EXP, scale=SCALE)
                                if sl == 0 or kb >= 8:
                                    mi = kb if sl == 0 else kb - 8
                                    nc.vector.tensor_mul(pt[:], pt[:], msk[:, mi, :])
                                if kb % 2 == 0:
                                    pt_prev = pt
                                else:
                                    pp = ptpool.tile([P, CH], BF16, tag="ptp", name="pp")
                                    nc.vector.tensor_add(pp[:], pt_prev[:], pt[:])
                                    if kb % 4 == 1:
                                        pp_prev = pp
                                    else:
                                        pq = ptpool.tile([P, CH], BF16, tag="ptq", name="pq")
                                        nc.vector.tensor_add(pq[:], pp_prev[:], pp[:])
                                        # slot-0 feeder has surplus yields:
                                        # pull extra fill so PE doesn't wait
                                        # on the DVE chain feeding pq
                                        if feeder is not None and sl == 0:
                                            for _ in range(2):
                                                next(feeder, None)
                                        nc.tensor.matmul(
                                            sums_ps[:], lhsT=ones_mat[:], rhs=pq[:],
                                            start=(kb == 3), stop=(kb == n_kb - 1),
                                        )
                                nc.tensor.matmul(
                                    oT_ps[:],
                                    lhsT=v4[:, kb, kv * P : (kv + 1) * P],
                                    rhs=pt[:],
                                    start=(kb == 0), stop=(kb == n_kb - 1),
                                )
                                if feeder is not None:
                                    next(feeder, None)
                            rb = ptpool.tile([P, CH], F32R, tag="ptr", name="rb")
                            with nc.allow_low_precision(reason="f32r softmax denom"):
                                nc.vector.reciprocal(rb[:], sums_ps[:])
                            osb = evpool.tile([P, CH], BF16, tag="evb", name="osb")
                            nc.vector.tensor_mul(osb[:], oT_ps[:], rb[:])
                            nc.sync.dma_start(out=oT_i[sl, h], in_=osb[:])
                    if feeder is not None:
                        for _ in feeder:
                            pass

            # ==== P3 slot 0 woven with the tail of P0 slot 1 ====
            attn_slot(0, p0_heads(1, 8, QH, 2, qrot_eng=nc.sync))
            p0c_cm.__exit__(None, None, None)
            p0s_cm.__exit__(None, None, None)
            qw_cm.__exit__(None, None, None)

            # ==== P3 slot 1 woven with P4 half 0; then P4 half 1 ====
            with tc.tile_pool(name="obp", bufs=1) as obp:
                ob_sb = obp.tile([P, D], F32, tag="ob")
                nc.sync.dma_start(
                    out=ob_sb[:], in_=ob.ap()[None, :].partition_broadcast(P)
                )
                with (
                    tc.tile_pool(name="p4a", bufs=1) as p4a,
                    tc.tile_pool(name="wb4a", bufs=2) as wb4a,
                ):
                    otr0 = p4a.tile([P, QH, CH], BF16, tag="ot0")
                    attn_slot(1, p4_half(0, 2, otr0, wb4a, ob_sb))
                with (
                    tc.tile_pool(name="p4b", bufs=1) as p4b,
                    tc.tile_pool(name="wb4b", bufs=3) as wb4b,
                ):
                    otr1 = p4b.tile([P, QH, CH], BF16, tag="ot1")
                    for _ in p4_half(1, 1 << 30, otr1, wb4b, ob_sb):
                        pass
            r_cm.__exit__(None, None, None)
            pt_cm.__exit__(None, None, None)
            qt_cm.__exit__(None, None, None)
            kv_cm.__exit__(None, None, None)

    nc.compile()
    return nc


def _get_nc():
    if "nc" not in _CACHE:
        _CACHE["nc"] = _build()
    return _CACHE["nc"]


_PERM = np.concatenate([np.arange(0, P, 2), np.arange(1, P, 2)])


def _prep_shared(qw_w, qw_b, kw_w, kw_b, vw_w, vw_b, ow_w, ow_b, fc, fs):
    f32 = np.float32
    c = np.ascontiguousarray
    # [h, dp, dt, fp] = w[h*128 + perm[fp], dt*128 + dp]
    qq = qw_w.reshape(QH, P, D)[:, _PERM, :]                      # [h, fp, d]
    qwT = c(qq.reshape(QH, P, ND, P).transpose(0, 3, 2, 1).astype(BF))
    kk = kw_w.reshape(KVH, P, D)[:, _PERM, :]
    kwT = c(kk.reshape(KVH, P, ND, P).transpose(0, 3, 2, 1).astype(BF))
    # per 4-head half: [dt, dp, j] = vw[hs*512 + j, dt*128 + dp]
    vwT = c(vw_w.reshape(2, 512, ND, P).transpose(0, 2, 3, 1).astype(BF))
    # [es, ft, fp, j] = ow[es*512 + j, ft*128 + fp]
    owT = c(ow_w.reshape(8, 512, ND, P).transpose(0, 2, 3, 1).astype(BF))
    cos_all = c(fc.T.astype(f32))  # [64, S]
    sin_all = c(fs.T.astype(f32))
    qbT = c(qw_b.reshape(QH, P)[:, _PERM].T.astype(f32))
    kbT = c(kw_b.reshape(KVH, P)[:, _PERM].T.astype(f32))
    return dict(
        qwT=qwT, kwT=kwT, vwT=vwT, owT=owT,
        cos_all=cos_all, sin_all=sin_all, qbT=qbT, kbT=kbT,
        vb=c(vw_b.astype(f32)), ob=c(ow_b.astype(f32)),
    )


def _masks_for(chunks):
    m = np.zeros((2, 8, P, CH), BF)
    kp = np.arange(P)[:, None]
    qi = np.arange(CH)[None, :]
    for sl in range(2):
        q0 = chunks[sl] * CH
        for mi in range(8):
            kb = mi if sl == 0 else mi + 8
            m[sl, mi] = (kb * P + kp <= q0 + qi).astype(BF)
    return m


def _core_chunks(core):
    b, par = core // 2, core % 2
    return b, ((0, 3) if par == 0 else (1, 2))


def _make_in_maps(inputs):
    """inputs: dict with the reference's setup_inputs() keys (numpy)."""
    g = lambda k: np.asarray(inputs[k])
    shared = _prep_shared(
        g("qw_w"), g("qw_b"), g("kw_w"), g("kw_b"), g("vw_w"), g("vw_b"),
        g("ow_w"), g("ow_b"), g("freqs_cos"), g("freqs_sin"),
    )
    input = g("input")
    in_maps = []
    for core in range(NCORES):
        b, chunks = _core_chunks(core)
        r = core % 2
        x = input[b].astype(np.float32)  # [S, D]
        # [s, dp, dt, t] = x[s*512 + t, dt*128 + dp]
        strips = np.ascontiguousarray(
            x.reshape(NCH, CH, ND, P).transpose(0, 3, 2, 1).astype(BF)
        )
        own = np.ascontiguousarray(strips[list(chunks)])
        cos_own = np.ascontiguousarray(
            np.stack([shared["cos_all"][:, c * CH : (c + 1) * CH] for c in chunks], 1)
        )
        sin_own = np.ascontiguousarray(
            np.stack([shared["sin_all"][:, c * CH : (c + 1) * CH] for c in chunks], 1)
        )
        vb_loc = shared["vb"][r * 512 : (r + 1) * 512]
        vbm = np.concatenate([
            vb_loc * (1.0 if r == 0 else 0.0),
            vb_loc * (1.0 if r == 1 else 0.0),
        ]).astype(np.float32)
        mm = np.zeros((P, 2), np.float32)
        mm[:, r] = 1.0
        m = dict(
            qwT=shared["qwT"], owT=shared["owT"],
            cos_all=shared["cos_all"], sin_all=shared["sin_all"],
            qbT=shared["qbT"], ob=shared["ob"],
            kwT_loc=np.ascontiguousarray(shared["kwT"][r * LKV : (r + 1) * LKV]),
            vwT_loc=np.ascontiguousarray(shared["vwT"][r]),
            kbT_loc=np.ascontiguousarray(shared["kbT"][:, r * LKV : (r + 1) * LKV]),
            vbm=vbm, mm=mm,
            ones=np.ones((P, P), BF),
            own_strips=own, in_strips=strips,
            cos_own=cos_own, sin_own=sin_own, masks=_masks_for(chunks),
        )
        in_maps.append(m)
    return in_maps


def kernel(input, freqs_cos, freqs_sin, qw_w, qw_b, kw_w, kw_b, vw_w, vw_b,
           ow_w, ow_b, start_pos):
    in_maps = _make_in_maps(dict(
        input=input, freqs_cos=freqs_cos, freqs_sin=freqs_sin,
        qw_w=qw_w, qw_b=qw_b, kw_w=kw_w, kw_b=kw_b, vw_w=vw_w, vw_b=vw_b,
        ow_w=ow_w, ow_b=ow_b,
    ))
    nc = _get_nc()
    res = run_bass_kernel_spmd(nc, in_maps, list(range(NCORES)))

    out = np.empty((B, S, D), np.float32)
    for core in range(NCORES):
        b, chunks = _core_chunks(core)
        r = res.results[core]["out"].reshape(2, CH, D)
        for sl in range(2):
            c0 = chunks[sl] * CH
            out[b, c0 : c0 + CH, :] = r[sl]
    return out


# revision 52
# speedup vs baseline: 1.0119x; 1.0024x over previous
"""GQA attention prefill (B=4, S=2048, D=4096, 32 q-heads / 8 kv-heads, rotary,
causal) on 8 TRN2 NeuronCores.

Sharding: token-parallel attention + tensor-parallel K/V projection.

Token side: core c handles batch c//2 and two 512-token chunks of its
sequence: chunks {0,3} for even cores, {1,2} for odd cores (zigzag splits the
causal triangle evenly). Each core computes the full Q projection for its
tokens, attention for all 32 heads over its tokens, and the output projection
for its tokens. Outputs are disjoint token slices, gathered on host.

K/V side: the 8 kv heads are tensor-parallel across the core PAIR that shares
a batch — role r = c%2 computes only kv heads {4r..4r+3} over the full
2048-token prefix (halves the K/V projection PE work). Each K/V tile is
written twice into a GLOBAL-layout DRAM buffer — once at the role-0 position
scaled by m0, once at the role-1 position scaled by m1, where (m0,m1) are
per-core {0,1} input scalars — so every core produces the same-layout buffer
with zeros in the partner's slots. The buffer is duplicated into two halves
(one DRAM->DRAM copy) and a pairwise ReduceScatter(add) (replica groups
[[0,1],[2,3],[4,5],[6,7]]) hands BOTH cores the fully merged global K/V at
the same address: zero+x = x, and both scatter halves are identical. The
SPMD program stays identical on all cores — no role-dependent addresses.

The collective fires right after the K/V pass and flies during the Q
projection (~380us of PE work), so its ~225us latency is hidden.

DMA-queue routing (each queue's sequencer serializes its DMAs through the
transfer, so dispatch is a per-queue resource):
  - sync (SP):    strips, odd q/v weights, v4/qt4/mask loads, oT/out writes
  - gpsimd (Pool): even q/v weights, kw, ow streams, the collective
  - scalar (Act):  K/V global-buffer writes, the D2D duplicate, qrot writes
                   (Act is otherwise idle outside attention)
  - vector (DVE):  kT loads for attention

Precision: projections bf16 x bf16 -> f32 PSUM; rotary applied on PSUM f32,
K/Q written back as bf16; softmax denominator via ones-vector matmul in PSUM
f32; AV and the output projection bf16.

Layout conventions:
  - activations for QK^T are kept transposed: [head_dim (partitions), tokens]
  - rotary pairs are de-interleaved (even dims -> partitions 0-63, odd ->
    64-127) via a host-side permutation of the qw/kw rows.
  - attention runs scores-transposed: ST[key, query] = kT.T @ qT, softmax over
    the partition (key) axis, denominator via ones-vector matmul, no
    max-subtraction (scores are O(1)).
"""

import numpy as np
import ml_dtypes

import concourse.bacc as bacc
import concourse.bass as bass
import concourse.tile as tile
from concourse import library_config, mybir
from concourse.bass_utils import run_bass_kernel_spmd

F32 = mybir.dt.float32
F32R = mybir.dt.float32r
BF16 = mybir.dt.bfloat16
EXP = mybir.ActivationFunctionType.Exp
ADD = mybir.AluOpType.add
MULT = mybir.AluOpType.mult

B, S, D = 4, 2048, 4096
QH, KVH, HEAD = 32, 8, 128
P = 128
CH = 512                # token chunk (= query tile)
NCH = S // CH           # 4 chunks per sequence
ND = D // P             # 32 d-tiles
LKV = 4                 # kv heads per core (tensor-parallel within pair)
NCORES = 8
NKB = (8, 16)           # key-blocks per query slot (padded, uniform)
SCALE = 1.0 / np.sqrt(HEAD)
BF = ml_dtypes.bfloat16
PAIRS = [[0, 1], [2, 3], [4, 5], [6, 7]]

_CACHE = {}


def _build():
    nc = bacc.Bacc("TRN2", target_bir_lowering=False, debug=False, num_devices=NCORES)

    # ---- per-core external inputs ----
    own = nc.dram_tensor("own_strips", [2, P, ND, CH], BF16, kind="ExternalInput")
    pref = nc.dram_tensor("in_strips", [NCH, P, ND, CH], BF16, kind="ExternalInput")
    qwT = nc.dram_tensor("qwT", [QH, P, ND, P], BF16, kind="ExternalInput")
    kwT = nc.dram_tensor("kwT_loc", [LKV, P, ND, P], BF16, kind="ExternalInput")
    vwT = nc.dram_tensor("vwT_loc", [ND, P, LKV * HEAD], BF16, kind="ExternalInput")
    owT = nc.dram_tensor("owT", [8, ND, P, 512], BF16, kind="ExternalInput")
    cos_own = nc.dram_tensor("cos_own", [64, 2, CH], F32, kind="ExternalInput")
    sin_own = nc.dram_tensor("sin_own", [64, 2, CH], F32, kind="ExternalInput")
    cos_all = nc.dram_tensor("cos_all", [64, S], F32, kind="ExternalInput")
    sin_all = nc.dram_tensor("sin_all", [64, S], F32, kind="ExternalInput")
    qbT = nc.dram_tensor("qbT", [P, QH], F32, kind="ExternalInput")
    kbT = nc.dram_tensor("kbT_loc", [P, LKV], F32, kind="ExternalInput")
    vbm = nc.dram_tensor("vbm", [2 * LKV * HEAD], F32, kind="ExternalInput")
    ob = nc.dram_tensor("ob", [D], F32, kind="ExternalInput")
    masks = nc.dram_tensor("masks", [2, 8, P, CH], BF16, kind="ExternalInput")
    ones = nc.dram_tensor("ones", [P, P], BF16, kind="ExternalInput")
    mm = nc.dram_tensor("mm", [P, 2], F32, kind="ExternalInput")

    # ---- internal DRAM ----
    # Global K/V layout (kind, row, P, 1024):
    #   kind 0 = kT: kv head j (GLOBAL), chunk tg -> row 2j + tg//2,
    #       cols (tg%2)*512.  (kT_g[j] = [128 hd, 2048 keys] as 2 rows)
    #   kind 1 = v: key-block kb -> row kb, cols j*128+hd (j GLOBAL kv head)
    # cc_in has two identical halves of this layout (masked: partner slots
    # zero); ReduceScatter(add) over the pair yields the merged layout cc_m
    # on both cores.
    cc_in = nc.dram_tensor("cc_in", [2, 2, 2 * KVH, P, 1024], BF16)
    cc_m = nc.dram_tensor("cc_m", [2, 2 * KVH, P, 1024], BF16)
    qT_i = nc.dram_tensor("qT_i", [2, QH, P, CH], BF16)
    oT_i = nc.dram_tensor("oT_i", [2, QH, P, CH], BF16)

    out = nc.dram_tensor("out", [8, P, D], F32, kind="ExternalOutput")

    with tile.TileContext(nc) as tc:
        nc.gpsimd.load_library(library_config.lib)
        with (
            tc.tile_pool(name="const", bufs=1) as const,
            tc.tile_pool(name="ev", bufs=3) as evpool,
            tc.tile_pool(name="rt", bufs=4) as rtpool,
            tc.tile_pool(name="ps", bufs=8, space="PSUM") as pspool,
        ):
            kbT_e = const.tile([64, LKV], F32, tag="kbte")
            kbT_o = const.tile([64, LKV], F32, tag="kbto")
            nc.scalar.dma_start(out=kbT_e[:], in_=kbT[0:64, :])
            nc.scalar.dma_start(out=kbT_o[:], in_=kbT[64:P, :])
            # all-ones stationary matrix: the denominator matmul writes the
            # key-sum replicated across ALL partitions, so the softmax tail
            # needs no partition_broadcast (no Pool-ring round trip).
            ones_mat = const.tile([P, P], BF16, tag="oc")
            nc.scalar.dma_start(out=ones_mat[:], in_=ones[:])
            mm_sb = const.tile([P, 2], F32, tag="mm")
            nc.scalar.dma_start(out=mm_sb[:], in_=mm[:])

            def rotary_evict(ps, dst, cos_ap, sin_ap, be, bo):
                """dst[0:64]=(pe+be)*cos-(po+bo)*sin; dst[64:128]=(pe+be)*sin+(po+bo)*cos"""
                pe, po = ps[0:64, :], ps[64:128, :]
                t1 = rtpool.tile([64, CH], F32, tag="rt", name="t1")
                t2 = rtpool.tile([64, CH], F32, tag="rt", name="t2")
                nc.vector.scalar_tensor_tensor(t1[:], pe, be, cos_ap, ADD, MULT)
                nc.vector.scalar_tensor_tensor(t2[:], po, bo, sin_ap, ADD, MULT)
                nc.vector.tensor_sub(dst[0:64, :], t1[:], t2[:])
                t3 = rtpool.tile([64, CH], F32, tag="rt", name="t3")
                t4 = rtpool.tile([64, CH], F32, tag="rt", name="t4")
                nc.vector.scalar_tensor_tensor(t3[:], pe, be, sin_ap, ADD, MULT)
                nc.vector.scalar_tensor_tensor(t4[:], po, bo, cos_ap, ADD, MULT)
                nc.vector.tensor_add(dst[64:128, :], t3[:], t4[:])

            p01_cm = tc.tile_pool(name="strip", bufs=3)
            strip_pool = p01_cm.__enter__()
            kw_cm = tc.tile_pool(name="kw", bufs=1)
            kwpool = kw_cm.__enter__()
            kw_tiles = {}

            # ============ P1: K/V projection, LOCAL kv heads only ============
            with tc.tile_pool(name="p1c", bufs=1) as p1c:
                cos_all_sb = p1c.tile([64, S], F32, tag="cosa")
                sin_all_sb = p1c.tile([64, S], F32, tag="sina")
                nc.sync.dma_start(out=cos_all_sb[:], in_=cos_all[:])
                nc.sync.dma_start(out=sin_all_sb[:], in_=sin_all[:])
                vbm_sb = p1c.tile([P, 2 * LKV * HEAD], F32, tag="vbm")
                nc.sync.dma_start(
                    out=vbm_sb[:], in_=vbm.ap()[None, :].partition_broadcast(P)
                )
                with tc.tile_pool(name="wb", bufs=6) as wbpool, \
                     tc.tile_pool(name="stg", bufs=1) as stgpool:
                    for pr in range(2):
                        strips = []
                        for i in range(2):
                            st = strip_pool.tile(
                                [P, ND, CH], BF16, tag="strip", name=f"strip{pr}_{i}"
                            )
                            # pr0's second strip rides the Act ring's idle
                            # start so both strips are up before the K pass;
                            # pr0's first strip loads in two halves so the
                            # first matmul can start after half the transfer
                            if pr == 0 and i == 0:
                                nc.sync.dma_start(
                                    out=st[:, 0 : ND // 2, :],
                                    in_=pref[0, :, 0 : ND // 2, :],
                                )
                                nc.sync.dma_start(
                                    out=st[:, ND // 2 :, :],
                                    in_=pref[0, :, ND // 2 :, :],
                                )
                            else:
                                eng = nc.scalar if (pr == 0 and i == 1) else nc.sync
                                eng.dma_start(out=st[:], in_=pref[2 * pr + i])
                            strips.append(st)
                        # K-pass (weight-stationary, out = kT [f, t]), 4 local
                        # heads. ts outer / kv inner: the 4 heads of one chunk
                        # land in consecutive global rows tg*4..tg*4+3, staged
                        # in SBUF and written per chunk as ONE 4-row DMA per
                        # scatter half (8x fewer ring-dispatch serializations).
                        for ts in range(2):
                            tg = 2 * pr + ts
                            ksg = stgpool.tile([P, LKV, 2, CH], BF16, tag="ksg",
                                               name="ksg")
                            for kv in range(LKV):
                                if kv in kw_tiles:
                                    w = kw_tiles[kv]
                                else:
                                    w = kwpool.tile([P, ND, P], BF16, tag=f"kw{kv}",
                                                    name=f"kw{kv}")
                                    nc.gpsimd.dma_start(out=w[:], in_=kwT[kv])
                                    kw_tiles[kv] = w
                                ps = pspool.tile([P, CH], F32, tag="ps", name="ps_k")
                                for dt in range(ND):
                                    nc.tensor.matmul(
                                        ps[:], lhsT=w[:, dt, :], rhs=strips[ts][:, dt, :],
                                        start=(dt == 0), stop=(dt == ND - 1),
                                    )
                                krot = evpool.tile([P, CH], F32, tag="ev", name="krot")
                                rotary_evict(
                                    ps, krot,
                                    cos_all_sb[:, tg * CH : (tg + 1) * CH],
                                    sin_all_sb[:, tg * CH : (tg + 1) * CH],
                                    kbT_e[:, kv : kv + 1], kbT_o[:, kv : kv + 1],
                                )
                                for q in range(2):
                                    nc.vector.tensor_scalar_mul(
                                        ksg[:, kv, q, :], krot[:], mm_sb[:, q : q + 1]
                                    )
                            for half, eng in ((0, nc.scalar), (1, nc.gpsimd)):
                                eng.dma_start(
                                    out=cc_in[half, 0, tg * LKV : (tg + 1) * LKV]
                                    .rearrange("r p t -> p r t"),
                                    in_=ksg[:],
                                )
                        # V-pass (input-stationary, out = v [t, hd]), 4 local heads
                        psv = [
                            pspool.tile([P, 512], F32, tag="ps", name=f"psv{i}")
                            for i in range(8)
                        ]
                        for dt in range(ND):
                            vw = wbpool.tile([P, 512], BF16, tag="wb", name="vw")
                            eng = nc.gpsimd if dt % 2 == 0 else nc.sync
                            eng.dma_start(out=vw[:], in_=vwT[dt])
                            for ts in range(2):
                                for tt in range(4):
                                    nc.tensor.matmul(
                                        psv[ts * 4 + tt][:],
                                        lhsT=strips[ts][:, dt, tt * P : (tt + 1) * P],
                                        rhs=vw[:],
                                        start=(dt == 0), stop=(dt == ND - 1),
                                    )
                        for ts in range(2):
                            kb0 = (2 * pr + ts) * 4
                            vsg = stgpool.tile([P, 4, 1024], BF16, tag="vsg", name="vsg")
                            for tt in range(4):
                                # masked double-write into both role column
                                # halves of the staged global v rows
                                for q in range(2):
                                    nc.vector.scalar_tensor_tensor(
                                        vsg[:, tt, q * 512 : (q + 1) * 512],
                                        psv[ts * 4 + tt][:],
                                        mm_sb[:, q : q + 1],
                                        vbm_sb[:, q * 512 : (q + 1) * 512],
                                        MULT, ADD,
                                    )
                            for half, eng in ((0, nc.scalar), (1, nc.gpsimd)):
                                eng.dma_start(
                                    out=cc_in[half, 1, kb0 : kb0 + 4]
                                    .rearrange("r p t -> p r t"),
                                    in_=vsg[:],
                                )

            kw_cm.__exit__(None, None, None)
            p01_cm.__exit__(None, None, None)

            # Pairwise ReduceScatter(add): both scatter halves are written
            # identically by the evictions, so every core receives the merged
            # global K/V at cc_m. Emitted before any Q-projection instruction
            # so the scheduler places it at the head of the Pool ring.
            nc.gpsimd.collective_compute(
                "ReduceScatter",
                mybir.AluOpType.add,
                replica_groups=PAIRS,
                ins=[cc_in.ap()],
                outs=[cc_m.ap()],
            )

            kv_cm = tc.tile_pool(name="kvS", bufs=1)
            kvpool = kv_cm.__enter__()
            qt_cm = tc.tile_pool(name="qtS", bufs=3)
            qtpool = qt_cm.__enter__()
            pt_cm = tc.tile_pool(name="ptS", bufs=4)
            ptpool = pt_cm.__enter__()
            r_cm = tc.tile_pool(name="rS", bufs=2)
            rpool = r_cm.__enter__()
            qw_cm = tc.tile_pool(name="qw", bufs=3)
            wpool = qw_cm.__enter__()

            # ============ P0: Q projection + rotary -> qT_i (bf16) ============
            # Slot 0 + 24 heads of slot 1 emitted eagerly (covers the
            # collective); the last 8 heads are woven into attention slot 0.
            p0s_cm = tc.tile_pool(name="p0strip", bufs=1)  # one buf per tag (2 tags)
            p0strip_pool = p0s_cm.__enter__()
            p0c_cm = tc.tile_pool(name="p0c", bufs=1)
            p0c = p0c_cm.__enter__()
            cos_own_sb = p0c.tile([64, 2, CH], F32, tag="coso")
            sin_own_sb = p0c.tile([64, 2, CH], F32, tag="sino")
            nc.sync.dma_start(out=cos_own_sb[:], in_=cos_own[:])
            nc.sync.dma_start(out=sin_own_sb[:], in_=sin_own[:])
            qbT_e = p0c.tile([64, QH], F32, tag="qbte")
            qbT_o = p0c.tile([64, QH], F32, tag="qbto")
            nc.sync.dma_start(out=qbT_e[:], in_=qbT[0:64, :])
            nc.sync.dma_start(out=qbT_o[:], in_=qbT[64:P, :])

            p0_strips = {}
            for _sl in range(2):
                _st = p0strip_pool.tile(
                    [P, ND, CH], BF16, tag=f"p0strip{_sl}", name=f"ostrip{_sl}"
                )
                nc.sync.dma_start(out=_st[:], in_=own[_sl])
                p0_strips[_sl] = _st

            def p0_heads(sl, h0, h1, group, qrot_eng=None):
                """Emit Q-proj for heads [h0,h1) of slot sl; yield after each
                `group` matmuls. Weight loads alternate Pool/SP queues."""
                qrot_eng = qrot_eng or nc.sync
                st = p0_strips[sl]
                for h in range(h0, h1):
                    w = wpool.tile([P, ND, P], BF16, tag="w", name=f"qw{sl}_{h}")
                    eng = (nc.gpsimd, nc.sync)[h % 2]
                    eng.dma_start(out=w[:], in_=qwT[h])
                    ps = pspool.tile([P, CH], F32, tag="ps", name="ps_q")
                    for dtg in range(ND // group):
                        for k in range(group):
                            dt = dtg * group + k
                            nc.tensor.matmul(
                                ps[:], lhsT=w[:, dt, :], rhs=st[:, dt, :],
                                start=(dt == 0), stop=(dt == ND - 1),
                            )
                        yield
                    qrot = evpool.tile([P, CH], BF16, tag="ev", name="qrot")
                    rotary_evict(
                        ps, qrot,
                        cos_own_sb[:, sl, :], sin_own_sb[:, sl, :],
                        qbT_e[:, h : h + 1], qbT_o[:, h : h + 1],
                    )
                    qrot_eng.dma_start(out=qT_i[sl, h], in_=qrot[:])

            for _ in p0_heads(0, 0, QH, ND):
                pass
            for _ in p0_heads(1, 0, 8, ND):
                pass

            def p4_half(hf, yield_every, otr, wb4pool, ob_sb):
                """Emit o-proj for token-slot half `hf` (ttiles 4hf..4hf+3)."""
                for hq in range(0, QH, 8):
                    nc.sync.dma_start(
                        out=otr[:, hq : hq + 8, :],
                        in_=oT_i[hf, hq : hq + 8].rearrange("h p t -> p h t"),
                    )
                for e in range(8):
                    ps4 = [
                        pspool.tile([P, 512], F32, tag="ps", name=f"ps4_{i}")
                        for i in range(4)
                    ]
                    cnt = 0
                    for f4 in range(ND // 4):
                        ow = wb4pool.tile([P, 4, 512], BF16, tag="wb4", name="ow")
                        nc.gpsimd.dma_start(
                            out=ow[:],
                            in_=owT[e, 4 * f4 : 4 * f4 + 4].rearrange("d p j -> p d j"),
                        )
                        for df in range(4):
                            ft = 4 * f4 + df
                            for tsub in range(4):
                                nc.tensor.matmul(
                                    ps4[tsub][:],
                                    lhsT=otr[:, ft, tsub * P : (tsub + 1) * P],
                                    rhs=ow[:, df, :],
                                    start=(ft == 0), stop=(ft == ND - 1),
                                )
                                cnt += 1
                                if cnt % yield_every == 0:
                                    yield
                    for tsub in range(4):
                        osb = evpool.tile([P, 512], F32, tag="ev4", name="osb4")
                        nc.vector.tensor_add(
                            osb[:], ps4[tsub][:], ob_sb[:, e * 512 : (e + 1) * 512]
                        )
                        nc.sync.dma_start(
                            out=out[hf * 4 + tsub, :, e * 512 : (e + 1) * 512],
                            in_=osb[:],
                        )

            def attn_slot(sl, feeder):
                n_kb = NKB[sl]
                nrow = n_kb // 8  # kT rows per head ([P,1024] each)
                with (
                    tc.tile_pool(name=f"mask{sl}", bufs=1) as mpool,
                    tc.tile_pool(name=f"v4{sl}", bufs=1) as v4pool,
                ):
                    msk = mpool.tile([P, 8, CH], BF16, tag="mask", name="msk")
                    nc.scalar.dma_start(
                        out=msk[:], in_=masks[sl].rearrange("m k q -> k m q")
                    )
                    # merged v for ALL 8 kv heads, whole slot: [keys, kb, 8*128]
                    # The slot-0 loads are scheduling-floored past the
                    # collective's completion so their collective-wait never
                    # stalls a DMA ring (whose completion counters gate
                    # unrelated queues via shared semaphore recycling).
                    v4 = v4pool.tile([P, n_kb, 1024], BF16, tag="v4", name="v4")
                    ld = nc.scalar if sl == 0 else nc.gpsimd
                    with tc.tile_wait_until(0.53, enable=(sl == 0)):
                        # merged kT for ALL 8 kv heads, whole slot, one DMA:
                        # slot 0 needs only the even rows (first 1024 keys).
                        ktall = kvpool.tile([P, 8 * nrow, 1024], BF16, tag="kt", name="ktall")
                        ld.dma_start(
                            out=ktall[:],
                            in_=cc_m[0, 0 : 8 * nrow].rearrange("r p t -> p r t"),
                        )
                        ld.dma_start(
                            out=v4[:, 0:n_kb, :],
                            in_=cc_m[1, 0:n_kb].rearrange("b p j -> p b j"),
                        )
                    for kv in range(KVH):
                        qt4 = qtpool.tile([P, 4, CH], BF16, tag="qt", name="qt4")
                        nc.sync.dma_start(
                            out=qt4[:],
                            in_=qT_i[sl, kv :: KVH].rearrange("g p t -> p g t"),
                        )
                        for g in range(4):
                            h = kv + KVH * g
                            oT_ps = pspool.tile([P, CH], F32, tag="ps", name="oT_ps")
                            sums_ps = pspool.tile([P, CH], F32, tag="ps", name="sums_ps")
                            for kb in range(n_kb):
                                st_ps = pspool.tile([P, CH], F32, tag="ps", name="st_ps")
                                nc.tensor.matmul(
                                    st_ps[:],
                                    lhsT=ktall[:, (kb // 4) * LKV + kv % LKV,
                                               (kv // LKV) * 512
                                               + (kb % 4) * P : (kv // LKV) * 512
                                               + (kb % 4 + 1) * P],
                                    rhs=qt4[:, g, :], start=True, stop=True,
                                )
                                pt = ptpool.tile([P, CH], BF16, tag="pt", name="pt")
                                nc.scalar.activation(pt[:], st_ps[:], EXP, scale=SCALE)
                                if sl == 0 or kb >= 8:
                                    mi = kb if sl == 0 else kb - 8
                                    nc.vector.tensor_mul(pt[:], pt[:], msk[:, mi, :])
                                if kb % 2 == 0:
                                    pt_prev = pt
                                else:
                                    pp = ptpool.tile([P, CH], BF16, tag="ptp", name="pp")
                                    nc.vector.tensor_add(pp[:], pt_prev[:], pt[:])
                                    if kb % 4 == 1:
                                        pp_prev = pp
                                    else:
                                        pq = ptpool.tile([P, CH], BF16, tag="ptq", name="pq")
                                        nc.vector.tensor_add(pq[:], pp_prev[:], pp[:])
                                        # slot-0 feeder has surplus yields:
                                        # pull extra fill so PE doesn't wait
                                        # on the DVE chain feeding pq
                                        if feeder is not None and sl == 0:
                                            for _ in range(2):
                                                next(feeder, None)
                                        nc.tensor.matmul(
                                            sums_ps[:], lhsT=ones_mat[:], rhs=pq[:],
                                            start=(kb == 3), stop=(kb == n_kb - 1),
                                        )
                                nc.tensor.matmul(
                                    oT_ps[:],
                                    lhsT=v4[:, kb, kv * P : (kv + 1) * P],
                                    rhs=pt[:],
                                    start=(kb == 0), stop=(kb == n_kb - 1),
                                )
                                if feeder is not None:
                                    next(feeder, None)
                            rb = ptpool.tile([P, CH], F32R, tag="ptr", name="rb")
                            with nc.allow_low_precision(reason="f32r softmax denom"):
                                nc.vector.reciprocal(rb[:], sums_ps[:])
                            osb = evpool.tile([P, CH], BF16, tag="evb", name="osb")
                            nc.vector.tensor_mul(osb[:], oT_ps[:], rb[:])
                            nc.sync.dma_start(out=oT_i[sl, h], in_=osb[:])
                    if feeder is not None:
                        for _ in feeder:
                            pass

            # ==== P3 slot 0 woven with the tail of P0 slot 1 ====
            attn_slot(0, p0_heads(1, 8, QH, 2, qrot_eng=nc.sync))
            p0c_cm.__exit__(None, None, None)
            p0s_cm.__exit__(None, None, None)
            qw_cm.__exit__(None, None, None)

            # ==== P3 slot 1 woven with P4 half 0; then P4 half 1 ====
            with tc.tile_pool(name="obp", bufs=1) as obp:
                ob_sb = obp.tile([P, D], F32, tag="ob")
                nc.sync.dma_start(
                    out=ob_sb[:], in_=ob.ap()[None, :].partition_broadcast(P)
                )
                with (
                    tc.tile_pool(name="p4a", bufs=1) as p4a,
                    tc.tile_pool(name="wb4a", bufs=2) as wb4a,
                ):
                    otr0 = p4a.tile([P, QH, CH], BF16, tag="ot0")
                    attn_slot(1, p4_half(0, 2, otr0, wb4a, ob_sb))
                with (
                    tc.tile_pool(name="p4b", bufs=1) as p4b,
                    tc.tile_pool(name="wb4b", bufs=3) as wb4b,
                ):
                    otr1 = p4b.tile([P, QH, CH], BF16, tag="ot1")
                    for _ in p4_half(1, 1 << 30, otr1, wb4b, ob_sb):
                        pass
            r_cm.__exit__(None, None, None)
            pt_cm.__exit__(None, None, None)
            qt_cm.__exit__(None, None, None)
            kv_cm.__exit__(None, None, None)

    nc.compile()
    return nc


def _get_nc():
    if "nc" not in _CACHE:
        _CACHE["nc"] = _build()
    return _CACHE["nc"]


_PERM = np.concatenate([np.arange(0, P, 2), np.arange(1, P, 2)])


def _prep_shared(qw_w, qw_b, kw_w, kw_b, vw_w, vw_b, ow_w, ow_b, fc, fs):
    f32 = np.float32
    c = np.ascontiguousarray
    # [h, dp, dt, fp] = w[h*128 + perm[fp], dt*128 + dp]
    qq = qw_w.reshape(QH, P, D)[:, _PERM, :]                      # [h, fp, d]
    qwT = c(qq.reshape(QH, P, ND, P).transpose(0, 3, 2, 1).astype(BF))
    kk = kw_w.reshape(KVH, P, D)[:, _PERM, :]
    kwT = c(kk.reshape(KVH, P, ND, P).transpose(0, 3, 2, 1).astype(BF))
    # per 4-head half: [dt, dp, j] = vw[hs*512 + j, dt*128 + dp]
    vwT = c(vw_w.reshape(2, 512, ND, P).transpose(0, 2, 3, 1).astype(BF))
    # [es, ft, fp, j] = ow[es*512 + j, ft*128 + fp]
    owT = c(ow_w.reshape(8, 512, ND, P).transpose(0, 2, 3, 1).astype(BF))
    cos_all = c(fc.T.astype(f32))  # [64, S]
    sin_all = c(fs.T.astype(f32))
    qbT = c(qw_b.reshape(QH, P)[:, _PERM].T.astype(f32))
    kbT = c(kw_b.reshape(KVH, P)[:, _PERM].T.astype(f32))
    return dict(
        qwT=qwT, kwT=kwT, vwT=vwT, owT=owT,
        cos_all=cos_all, sin_all=sin_all, qbT=qbT, kbT=kbT,
        vb=c(vw_b.astype(f32)), ob=c(ow_b.astype(f32)),
    )


def _masks_for(chunks):
    m = np.zeros((2, 8, P, CH), BF)
    kp = np.arange(P)[:, None]
    qi = np.arange(CH)[None, :]
    for sl in range(2):
        q0 = chunks[sl] * CH
        for mi in range(8):
            kb = mi if sl == 0 else mi + 8
            m[sl, mi] = (kb * P + kp <= q0 + qi).astype(BF)
    return m


def _core_chunks(core):
    b, par = core // 2, core % 2
    return b, ((0, 3) if par == 0 else (1, 2))


def _make_in_maps(inputs):
    """inputs: dict with the reference's setup_inputs() keys (numpy)."""
    g = lambda k: np.asarray(inputs[k])
    shared = _prep_shared(
        g("qw_w"), g("qw_b"), g("kw_w"), g("kw_b"), g("vw_w"), g("vw_b"),
        g("ow_w"), g("ow_b"), g("freqs_cos"), g("freqs_sin"),
    )
    input = g("input")
    in_maps = []
    for core in range(NCORES):
        b, chunks = _core_chunks(core)
        r = core % 2
        x = input[b].astype(np.float32)  # [S, D]
        # [s, dp, dt, t] = x[s*512 + t, dt*128 + dp]
        strips = np.ascontiguousarray(
            x.reshape(NCH, CH, ND, P).transpose(0, 3, 2, 1).astype(BF)
        )
        own = np.ascontiguousarray(strips[list(chunks)])
        cos_own = np.ascontiguousarray(
            np.stack([shared["cos_all"][:, c * CH : (c + 1) * CH] for c in chunks], 1)
        )
        sin_own = np.ascontiguousarray(
            np.stack([shared["sin_all"][:, c * CH : (c + 1) * CH] for c in chunks], 1)
        )
        vb_loc = shared["vb"][r * 512 : (r + 1) * 512]
        vbm = np.concatenate([
            vb_loc * (1.0 if r == 0 else 0.0),
            vb_loc * (1.0 if r == 1 else 0.0),
        ]).astype(np.float32)
        mm = np.zeros((P, 2), np.float32)
        mm[:, r] = 1.0
        m = dict(
            qwT=shared["qwT"], owT=shared["owT"],
            cos_all=shared["cos_all"], sin_all=shared["sin_all"],
            qbT=shared["qbT"], ob=shared["ob"],
            kwT_loc=np.ascontiguousarray(shared["kwT"][r * LKV : (r + 1) * LKV]),
            vwT_loc=np.ascontiguousarray(shared["vwT"][r]),
            kbT_loc=np.ascontiguousarray(shared["kbT"][:, r * LKV : (r + 1) * LKV]),
            vbm=vbm, mm=mm,
            ones=np.ones((P, P), BF),
            own_strips=own, in_strips=strips,
            cos_own=cos_own, sin_own=sin_own, masks=_masks_for(chunks),
        )
        in_maps.append(m)
    return in_maps


def kernel(input, freqs_cos, freqs_sin, qw_w, qw_b, kw_w, kw_b, vw_w, vw_b,
           ow_w, ow_b, start_pos):
    in_maps = _make_in_maps(dict(
        input=input, freqs_cos=freqs_cos, freqs_sin=freqs_sin,
        qw_w=qw_w, qw_b=qw_b, kw_w=kw_w, kw_b=kw_b, vw_w=vw_w, vw_b=vw_b,
        ow_w=ow_w, ow_b=ow_b,
    ))
    nc = _get_nc()
    res = run_bass_kernel_spmd(nc, in_maps, list(range(NCORES)))

    out = np.empty((B, S, D), np.float32)
    for core in range(NCORES):
        b, chunks = _core_chunks(core)
        r = res.results[core]["out"].reshape(2, CH, D)
        for sl in range(2):
            c0 = chunks[sl] * CH
            out[b, c0 : c0 + CH, :] = r[sl]
    return out
